# revision 16
# baseline (speedup 1.0000x reference)
"""MoE feed-forward (8 experts, top-2) on 8 TRN2 NeuronCores, expert-parallel.

v2.2: latency-oriented restructure of the 956us baseline.

Per-core pipeline (core c owns expert c):
  A. fp32 gating from host-pretransposed xT (gate_w-stationary PE matmuls),
     top-2 via DVE max/max_index, sigmoid weights; per-group triangular-
     matmul prefix sums assign [expert, C_g+1] send slots; bf16 x rows come
     pre-cast from the host. xT streams in first across four DMA queues so
     gating starts ~12us in; expert weights queue behind it.
  B. Dispatch split into two AllToAlls over token groups (0-255, 256-1023):
     the small first group fires ~25us in, absorbing inter-core start skew,
     and its chunk row C_g carries the per-expert counts so the receive-side
     slot tables build with no extra collective.
  C. Expert MLP per 512-row tile, fed by dma_gather(transpose=True) straight
     from the padded recv buffer into xrT [128, d-blk, tok] bf16 layout.
     w1 runs two interleaved PSUM accumulation chains (m-block pairs) and w2
     alternates its two d-chunk banks per m so no two consecutive matmuls
     accumulate into the same PSUM bank.
  D. y rows scatter (indirect, OOB-clipped) back into recv-slot order; one
     return A2A per group so most of the return wire time hides under
     compute.
  E. Combine: indirect gathers of each token's two expert rows + weighted
     add, fp32 out.
"""
import numpy as np
import ml_dtypes

import concourse.bass as bass
import concourse.mybir as mybir
import concourse.tile as tile
from concourse import bacc
from concourse.bass import IndirectOffsetOnAxis
from concourse.bass_utils import run_bass_kernel_spmd
from concourse.masks import make_identity, make_upper_triangular

D_MODEL, HIDDEN, N_EXPERTS, TOP_K = 1024, 4096, 8, 2
N_CORES = 8
P = 128
T = 8192
T_LOC = T // N_CORES            # 1024 tokens per core
N_TOK_TILES = T_LOC // P        # 8
D_BLKS = D_MODEL // P           # 8
H_BLKS = HIDDEN // P            # 32
GROUPS = [(0, 2), (2, 5), (7, 1)]   # (first token tile, n tiles) per group

FP32 = mybir.dt.float32
BF16 = mybir.dt.bfloat16
I32 = mybir.dt.int32
U32 = mybir.dt.uint32
AF = mybir.ActivationFunctionType
ALU = mybir.AluOpType

RG = [list(range(N_CORES))]


def _dram_alias(nc, base_handle, name):
    """A DRAM tensor handle aliasing base_handle's memory. Distinct names keep
    Tile's conservative same-tensor WAW tracking from serializing writers that
    are known (by construction) to touch disjoint rows."""
    mls = nc._tensor(name, list(base_handle.shape), base_handle.dtype,
                     kind="Internal", type="DRAM")
    base_mloc = nc.lookup_mloc(base_handle)
    mloc = mls.memorylocations[0]
    mloc.allocated = base_mloc.allocated
    mloc.addr = base_mloc.addr
    return bass.DRamTensorHandle(name, list(base_handle.shape),
                                 base_handle.dtype)


def _tiles_of(total, big=512):
    out = []
    off = 0
    while off < total:
        nt = min(big, total - off)
        out.append((off, nt))
        off += nt
    return out


def _body(tc, CH, SDEV, xT_loc, x_rows_loc, gate_w, gate_b_rep, iota8_rep,
          w1_loc, b1_loc, w2_loc, b2_rep, riota_w, idid16, riota_cols,
          out_loc):
    """CH = per-group slot capacities (chunk rows are C+1; row C is counts).
    SDEV = per-group 128-multiple compute row counts."""
    nc = tc.nc
    NG = len(GROUPS)
    SROWS = [N_EXPERTS * (c + 1) for c in CH]

    send_x_t = [nc.dram_tensor(f"send_x{h}", [SROWS[h], D_MODEL], BF16)
                for h in range(NG)]
    send_x_al = [[_dram_alias(nc, send_x_t[h], f"sx{h}_al{i}")
                  for i in range(GROUPS[h][1] * TOP_K + 1)] for h in range(NG)]
    send_y_t = [nc.dram_tensor(f"send_y{h}", [SROWS[h], D_MODEL], BF16)
                for h in range(NG)]
    send_y_al = [[_dram_alias(nc, send_y_t[h], f"sy{h}_al{i}")
                  for i in range(SDEV[h] // P)] for h in range(NG)]

    with tc.tile_pool(name="dram", bufs=1, space="DRAM") as dram, \
         tc.tile_pool(name="persist", bufs=1) as persist:
        recv_x = [dram.tile([SROWS[h], D_MODEL], BF16, name=f"recv_x{h}",
                            tag=f"recv_x{h}") for h in range(NG)]
        recv_y = [dram.tile([SROWS[h], D_MODEL], BF16, name=f"recv_y{h}",
                            tag=f"recv_y{h}") for h in range(NG)]

        ident = persist.tile([P, P], FP32)
        make_identity(nc, ident)
        strictu = persist.tile([P, P], FP32)
        make_upper_triangular(nc, strictu, val=1.0, diag=False)
        ones_t = persist.tile([P, P], FP32)
        nc.gpsimd.memset(ones_t, 1.0)

        gb_sb = persist.tile([P, N_EXPERTS], FP32)
        nc.scalar.dma_start(gb_sb, gate_b_rep[:])
        iota_sb = persist.tile([P, N_EXPERTS], FP32)
        nc.scalar.dma_start(iota_sb, iota8_rep[:])
        gw_sb = persist.tile([P, D_BLKS, N_EXPERTS], FP32)
        nc.scalar.dma_start(gw_sb, gate_w[:].rearrange("(j p) e -> p j e", p=P))
        b1_sb = persist.tile([P, H_BLKS], FP32)
        nc.scalar.dma_start(b1_sb, b1_loc[:])
        b2r_sb = persist.tile([P, D_MODEL], FP32)
        nc.scalar.dma_start(b2r_sb, b2_rep[:])
        riw_sb = persist.tile([P, max(SDEV) // 16], FP32)
        nc.scalar.dma_start(riw_sb, riota_w[:])
        ric_sb = persist.tile([P, max(SDEV) // P], FP32)
        nc.scalar.dma_start(ric_sb, riota_cols[:])

        wts_sb = persist.tile([P, N_TOK_TILES, TOP_K], FP32)
        rows32 = persist.tile([P, N_TOK_TILES, TOP_K], I32)
        sendmask = persist.tile([P, N_TOK_TILES * N_EXPERTS], FP32)
        gth_sb = persist.tile([P, NG, max(SDEV) // 16], mybir.dt.int16)
        ret_sb = persist.tile([P, NG, max(SDEV) // P], I32)

        w1_sb = persist.tile([P, D_BLKS, HIDDEN], BF16)
        w2_sb = persist.tile([P, H_BLKS, D_MODEL], BF16)

        # =========== phase A: gating + routing + dispatch scatter ===========
        a2a_x = []
        with tc.tile_pool(name="phA", bufs=2) as pA, \
             tc.tile_pool(name="phA1", bufs=1) as pA1, \
             tc.tile_pool(name="phA_psum", bufs=2, space="PSUM") as pAp:
            # xT first, split across the three DMA queues, so gating starts
            # early; x row copies next; the 16MB weight stream queues after.
            qs = [nc.scalar, nc.gpsimd, nc.sync]
            xT_sb = pA1.tile([P, D_BLKS, T_LOC], FP32)
            xT_insts = []
            for j in range(D_BLKS):
                xi = qs[j % 3].dma_start(xT_sb[:, j, :],
                                         xT_loc[j * P:(j + 1) * P, :])
                xT_insts.append(xi)
            x_rows = pA1.tile([P, N_TOK_TILES, D_MODEL], BF16)
            for i in range(N_TOK_TILES):
                qs[i % 3].dma_start(x_rows[:, i, :],
                                    x_rows_loc[i * P:(i + 1) * P, :])
            for j in range(D_BLKS):
                wi = nc.sync.dma_start(w1_sb[:, j, :],
                                       w1_loc[j * P:(j + 1) * P, :])
                if j == 0:
                    for xi in xT_insts:
                        bass._add_dep_helper(wi.ins, xi.ins, sync=True,
                                             reason="weights after xT")
            for m in range(H_BLKS):
                nc.sync.dma_start(w2_sb[:, m, :],
                                  w2_loc[m * P:(m + 1) * P, :])

            logitc = pA1.tile([P, N_TOK_TILES, N_EXPERTS], FP32)
            maxcat = pA1.tile([P, N_TOK_TILES, 8], FP32)
            idxcat8 = pA1.tile([P, N_TOK_TILES, 8], U32)
            idxfcat = pA1.tile([P, N_TOK_TILES, TOP_K], FP32)
            offs = pA1.tile([P, N_TOK_TILES, N_EXPERTS], FP32)
            csum_sb = pA1.tile([P, N_TOK_TILES * N_EXPERTS], FP32)
            gated = [False, False]

            def emit_gate_chunk(tc2):
                gps = pAp.tile([8, 512], FP32, tag="gps", name="gps")
                for j in range(D_BLKS):
                    nc.tensor.matmul(gps, lhsT=gw_sb[:, j, :],
                                     rhs=xT_sb[:, j, tc2 * 512:(tc2 + 1) * 512],
                                     start=(j == 0), stop=(j == D_BLKS - 1))
                logit8 = pA.tile([8, 512], FP32, tag="lg8", name="lg8")
                nc.vector.tensor_copy(logit8, gps)
                gated[tc2] = logit8

            def emit_tile_top2(i):
                logit8 = gated[i // 4]
                ii = i % 4
                tp = pAp.tile([P, 8], FP32, tag="tp", name="tp", bufs=2)
                nc.tensor.transpose(tp, logit8[:, ii * P:(ii + 1) * P],
                                    ident[0:8, 0:8])
                nc.vector.tensor_add(logitc[:, i, :], tp, gb_sb)
                nc.vector.max(maxcat[:, i, :], logitc[:, i, :])
                nc.vector.max_index(idxcat8[:, i, :], maxcat[:, i, :],
                                    logitc[:, i, :])

            emit_gate_chunk(0)
            for i in range(0, 4):
                emit_tile_top2(i)

            for h, (base, HB) in enumerate(GROUPS):
                C = CH[h]
                sl = slice(base * N_EXPERTS, (base + HB) * N_EXPERTS)
                hsl = slice(base, base + HB)

                d01 = pA.tile([P, HB], FP32, tag=f"d01{h}", name=f"d01{h}")
                nc.vector.tensor_sub(d01, maxcat[:, hsl, 0],
                                     maxcat[:, hsl, 1])
                nc.scalar.activation(wts_sb[:, hsl, 0], d01, AF.Sigmoid)
                nc.scalar.activation(wts_sb[:, hsl, 1], d01, AF.Sigmoid,
                                     scale=-1.0)
                nc.vector.tensor_copy(idxfcat[:, hsl, :],
                                      idxcat8[:, hsl, 0:TOP_K])
                eqc = [None, None]
                for k in range(TOP_K):
                    eqk = pA.tile([P, HB, N_EXPERTS], FP32, tag=f"eq{k}{h}",
                                  name=f"eq{k}{h}")
                    nc.vector.tensor_tensor(
                        eqk,
                        idxfcat[:, hsl, k:k + 1].to_broadcast(
                            [P, HB, N_EXPERTS]),
                        iota_sb[:, None, :].to_broadcast([P, HB, N_EXPERTS]),
                        op=ALU.is_equal)
                    eqc[k] = eqk
                nc.vector.tensor_add(
                    sendmask[:].rearrange("p (i e) -> p i e", e=N_EXPERTS)
                    [:, hsl, :], eqc[0], eqc[1])

                pref_ps = pAp.tile([P, HB * N_EXPERTS], FP32, tag="pref",
                                   name="pref", bufs=2)
                nc.tensor.matmul(pref_ps, lhsT=strictu, rhs=sendmask[:, sl],
                                 start=True, stop=True)
                csum_ps = pAp.tile([P, HB * N_EXPERTS], FP32, tag="csum",
                                   name="csum", bufs=2)
                nc.tensor.matmul(csum_ps, lhsT=ones_t, rhs=sendmask[:, sl],
                                 start=True, stop=True)
                offs_flat = offs[:].rearrange("p a b -> p (a b)")
                nc.vector.tensor_copy(offs_flat[:, sl], pref_ps)
                nc.vector.tensor_copy(csum_sb[:, sl], csum_ps)
                for i in range(base + 1, base + HB):
                    prev = csum_sb[:, (i - 1) * N_EXPERTS:i * N_EXPERTS]
                    nc.vector.tensor_add(offs[:, i, :], offs[:, i, :], prev)
                    cur = csum_sb[:, i * N_EXPERTS:(i + 1) * N_EXPERTS]
                    nc.vector.tensor_add(cur, cur, prev)

                rowsf = pA.tile([P, HB, TOP_K], FP32, tag=f"rowsf{h}",
                                name=f"rowsf{h}")
                nc.vector.tensor_scalar(rowsf, idxfcat[:, hsl, :],
                                        float(C + 1), None, op0=ALU.mult)
                for k in range(TOP_K):
                    prod = pA.tile([P, HB, N_EXPERTS], FP32, tag=f"prod{h}",
                                   name=f"prod{h}")
                    nc.vector.tensor_mul(prod, offs[:, hsl, :], eqc[k])
                    slotk = pA.tile([P, HB], FP32, tag=f"slotk{h}",
                                    name=f"slotk{h}")
                    nc.vector.reduce_sum(slotk, prod,
                                         axis=mybir.AxisListType.X)
                    nc.vector.tensor_add(rowsf[:, :, k], rowsf[:, :, k],
                                         slotk)
                nc.vector.tensor_copy(rows32[:, hsl, :], rowsf)

                # counts -> row C of every expert chunk (16 bf16 elems each)
                cntw = pA.tile([1, N_EXPERTS, 16], BF16, tag=f"cntw{h}",
                               name=f"cntw{h}")
                last = csum_sb[0:1, (base + HB - 1) * N_EXPERTS:
                               (base + HB) * N_EXPERTS]
                nc.vector.tensor_copy(
                    cntw, last[:, :, None].to_broadcast([1, N_EXPERTS, 16]))
                scatter_insts = []
                si = nc.scalar.dma_start(
                    send_x_al[h][HB * TOP_K].ap()[:]
                    .rearrange("(e r) d -> e r d", r=C + 1)[:, C, 0:16],
                    cntw[:])
                scatter_insts.append(si)

                for ii in range(HB):
                    i = base + ii
                    for k in range(TOP_K):
                        si = nc.gpsimd.indirect_dma_start(
                            out=send_x_al[h][ii * TOP_K + k].ap(),
                            out_offset=IndirectOffsetOnAxis(
                                ap=rows32[:, i, k:k + 1], axis=0),
                            in_=x_rows[:, i, :],
                            in_offset=None,
                            bounds_check=SROWS[h] - 1, oob_is_err=False)
                        scatter_insts.append(si)

                if h < NG - 1:
                    cc = nc.gpsimd.collective_compute(
                        "AllToAll", ALU.bypass, replica_groups=RG,
                        ins=[send_x_t[h].ap()[:].opt()],
                        outs=[recv_x[h][:].opt()])
                    for si in scatter_insts:
                        bass._add_dep_helper(cc.ins, si.ins, sync=True,
                                             reason="a2a after aliased scatters")
                    a2a_x.append(cc)
                else:
                    last_scatters = scatter_insts

                if h == 0:
                    emit_gate_chunk(1)
                    for i in range(2, N_TOK_TILES):
                        emit_tile_top2(i)

        # ====== phases B-D: per-group slot tables + expert MLP + return ======
        # Group 0 computes the FULL padded recv buffer with compile-time
        # identity indices: its first gather fires the moment A2A#0 lands,
        # with no count/table dependency. Later groups' tables are emitted
        # inside the previous group's first tile so they build mid-compute.
        a2a_y = []
        with tc.tile_pool(name="phC", bufs=2) as pC, \
             tc.tile_pool(name="phCh", bufs=1) as pCh, \
             tc.tile_pool(name="phC_psum", bufs=4, space="PSUM") as pCp:
            hT = pCh.tile([P, H_BLKS, 512], BF16)
            early_gathers = []
            idid_sb = pC.tile([P, SROWS[0] // 16], mybir.dt.int16,
                              tag="idid", name="idid", bufs=1)
            nc.scalar.dma_start(idid_sb, idid16[:])
            ret0 = pC.tile([P, SROWS[0] // P], I32, tag="ret0", name="ret0",
                           bufs=1)
            nc.vector.tensor_copy(ret0, ric_sb[:, 0:SROWS[0] // P])

            def emit_table(h):
                C = CH[h]
                S = SROWS[h]
                NW = SDEV[h] // 16
                NCOL = SDEV[h] // P
                cnt128 = pC.tile([1, N_EXPERTS * 16], BF16, tag="cnt128",
                                 name="cnt128", bufs=1)
                nc.scalar.dma_start(
                    cnt128,
                    recv_x[h][:].rearrange("(e r) d -> e r d", r=C + 1)
                    [:, C, 0:16])
                cnt2 = pC.tile([1, 2 * N_EXPERTS], FP32, tag="cnt2",
                               name="cnt2", bufs=1)
                cnt_row = cnt2[:, 0:N_EXPERTS]
                cum_row = cnt2[:, N_EXPERTS:]
                cnt_v = cnt128[:].rearrange("o (e u) -> o e u", u=16)[:, :, 0]
                nc.vector.tensor_copy(cnt_row, cnt_v)
                nc.vector.tensor_copy(cum_row, cnt_v)
                for s in range(1, N_EXPERTS):
                    nc.vector.tensor_add(cum_row[:, s:s + 1],
                                         cum_row[:, s:s + 1],
                                         cum_row[:, s - 1:s])
                bc = pCp.tile([P, 2 * N_EXPERTS], FP32, tag="bc", name="bc",
                              bufs=1)
                nc.tensor.matmul(bc, lhsT=ones_t[0:1, :], rhs=cnt2[:],
                                 start=True, stop=True)
                cntb = pC.tile([P, 2 * N_EXPERTS], FP32, tag="cntb",
                               name="cntb", bufs=1)
                nc.vector.tensor_copy(cntb, bc)

                def slot_table(out_ap, iota_ap, np_, tag, clamp):
                    msk3 = pC.tile([P, np_, N_EXPERTS], FP32, tag=tag + "m3",
                                   name=tag + "m3", bufs=1)
                    nc.vector.tensor_tensor(
                        msk3,
                        iota_ap[:, :, None].to_broadcast([P, np_, N_EXPERTS]),
                        cntb[:, None, N_EXPERTS:2 * N_EXPERTS]
                        .to_broadcast([P, np_, N_EXPERTS]),
                        op=ALU.is_ge)
                    s_of = pC.tile([P, np_], FP32, tag=tag + "s",
                                   name=tag + "s", bufs=1)
                    nc.vector.reduce_sum(s_of, msk3[:],
                                         axis=mybir.AxisListType.X)
                    nc.vector.tensor_tensor(
                        msk3, msk3[:],
                        cntb[:, None, 0:N_EXPERTS].to_broadcast(
                            [P, np_, N_EXPERTS]),
                        op=ALU.mult)
                    cume = pC.tile([P, np_], FP32, tag=tag + "c",
                                   name=tag + "c", bufs=1)
                    nc.vector.reduce_sum(cume, msk3[:],
                                         axis=mybir.AxisListType.X)
                    gf = pC.tile([P, np_], FP32, tag=tag + "g",
                                 name=tag + "g", bufs=1)
                    nc.vector.tensor_scalar(gf, s_of, float(C + 1), None,
                                            op0=ALU.mult)
                    nc.vector.tensor_add(gf, gf, iota_ap)
                    nc.vector.tensor_sub(gf, gf, cume)
                    if clamp:
                        nc.vector.tensor_scalar(gf, gf, float(S - 1), None,
                                                op0=ALU.min)
                    nc.vector.tensor_copy(out_ap, gf)

                slot_table(gth_sb[:, h, 0:NW], riw_sb[:, 0:NW], NW,
                           f"gw{h}", True)
                slot_table(ret_sb[:, h, 0:NCOL], ric_sb[:, 0:NCOL], NCOL,
                           f"rc{h}", False)

            for h in range(NG):
                S = SROWS[h]
                ret_insts = []
                for ti, (r0, NT) in enumerate(_tiles_of(SDEV[h])):
                    xrT = pC.tile([P, D_BLKS, NT], BF16, tag=f"xrT{NT}",
                                  name=f"xrT{NT}", bufs=2 if NT == 512 else 1)
                    idx_ap = (idid_sb[:, r0 // 16:(r0 + NT) // 16] if h == 0
                              else gth_sb[:, h, r0 // 16:(r0 + NT) // 16])
                    gi = nc.gpsimd.dma_gather(
                        xrT[:], recv_x[h][:], idx_ap,
                        NT, NT, D_MODEL, transpose=True)
                    if h == 0 and ti <= 1:
                        early_gathers.append(gi)
                    if h == 0 and ti == 1:
                        # last group's dispatch A2A, deferred so its
                        # completion-serialized trigger doesn't block the
                        # first compute gathers on the gpsimd queue
                        ccl = nc.gpsimd.collective_compute(
                            "AllToAll", ALU.bypass, replica_groups=RG,
                            ins=[send_x_t[NG - 1].ap()[:].opt()],
                            outs=[recv_x[NG - 1][:].opt()])
                        for si in last_scatters:
                            bass._add_dep_helper(ccl.ins, si.ins, sync=True,
                                                 reason="a2a after scatters")
                        for gi2 in early_gathers:
                            bass._add_dep_helper(ccl.ins, gi2.ins, sync=True,
                                                 reason="trigger after gathers")
                        a2a_x.append(ccl)
                    for mp in range(0, H_BLKS, 2):
                        psA = pCp.tile([P, 512], FP32, tag="ps1",
                                       name="ps1", bufs=4)
                        psB = pCp.tile([P, 512], FP32, tag="ps1",
                                       name="ps1", bufs=4)
                        for j in range(D_BLKS):
                            nc.tensor.matmul(
                                psA[:, 0:NT],
                                lhsT=w1_sb[:, j, mp * P:(mp + 1) * P],
                                rhs=xrT[:, j, :],
                                start=(j == 0), stop=(j == D_BLKS - 1),
                                skip_group_check=True)
                            nc.tensor.matmul(
                                psB[:, 0:NT],
                                lhsT=w1_sb[:, j, (mp + 1) * P:(mp + 2) * P],
                                rhs=xrT[:, j, :],
                                start=(j == 0), stop=(j == D_BLKS - 1),
                                skip_group_check=True)
                        nc.scalar.activation(hT[:, mp, 0:NT], psA[:, 0:NT],
                                             AF.Silu, bias=b1_sb[:, mp:mp + 1])
                        nc.scalar.activation(hT[:, mp + 1, 0:NT],
                                             psB[:, 0:NT], AF.Silu,
                                             bias=b1_sb[:, mp + 1:mp + 2])
                    if ti == 0 and h + 1 < NG:
                        # next group's tables build during this tile's w2
                        emit_table(h + 1)
                    for t in range(NT // P):
                        col = r0 // P + t
                        y_tm = pC.tile([P, D_MODEL], BF16, tag="y_tm",
                                       name="y_tm", bufs=2)
                        ps2 = [pCp.tile([P, 512], FP32, tag="ps2",
                                        name="ps2", bufs=2)
                               for _ in range(2)]
                        for m in range(H_BLKS):
                            for nh in range(2):
                                nc.tensor.matmul(
                                    ps2[nh], lhsT=hT[:, m, t * P:(t + 1) * P],
                                    rhs=w2_sb[:, m, nh * 512:(nh + 1) * 512],
                                    start=(m == 0), stop=(m == H_BLKS - 1),
                                    skip_group_check=True)
                        for nh in range(2):
                            nc.vector.tensor_add(
                                y_tm[:, nh * 512:(nh + 1) * 512], ps2[nh],
                                b2r_sb[:, nh * 512:(nh + 1) * 512])
                        roff = (ret0[:, col:col + 1] if h == 0
                                else ret_sb[:, h, col:col + 1])
                        si = nc.gpsimd.indirect_dma_start(
                            out=send_y_al[h][col].ap(),
                            out_offset=IndirectOffsetOnAxis(ap=roff, axis=0),
                            in_=y_tm[:],
                            in_offset=None,
                            bounds_check=S - 1, oob_is_err=False)
                        ret_insts.append(si)

                cc = nc.gpsimd.collective_compute(
                    "AllToAll", ALU.bypass, replica_groups=RG,
                    ins=[send_y_t[h].ap()[:].opt()],
                    outs=[recv_y[h][:].opt()])
                for si in ret_insts:
                    bass._add_dep_helper(cc.ins, si.ins, sync=True,
                                         reason="ret a2a after aliased scatters")
                a2a_y.append(cc)

        # =========== phase E: gather + weighted combine ===========
        with tc.tile_pool(name="phE", bufs=2) as pE:
            for h, (base, HB) in enumerate(GROUPS):
                for i in range(base, base + HB):
                    g0 = pE.tile([P, D_MODEL], BF16, tag="g0", name="g0")
                    nc.gpsimd.indirect_dma_start(
                        out=g0, out_offset=None, in_=recv_y[h][:],
                        in_offset=IndirectOffsetOnAxis(ap=rows32[:, i, 0:1],
                                                       axis=0))
                    g1 = pE.tile([P, D_MODEL], BF16, tag="g1", name="g1")
                    nc.gpsimd.indirect_dma_start(
                        out=g1, out_offset=None, in_=recv_y[h][:],
                        in_offset=IndirectOffsetOnAxis(ap=rows32[:, i, 1:2],
                                                       axis=0))
                    t0 = pE.tile([P, D_MODEL], FP32, tag="t0", name="t0")
                    nc.vector.tensor_scalar_mul(t0, g0, wts_sb[:, i, 0:1])
                    t1 = pE.tile([P, D_MODEL], FP32, tag="t1", name="t1")
                    nc.vector.tensor_scalar_mul(t1, g1, wts_sb[:, i, 1:2])
                    out_t = pE.tile([P, D_MODEL], FP32, tag="out_t",
                                    name="out_t")
                    nc.vector.tensor_add(out_t, t0, t1)
                    nc.scalar.dma_start(out_loc[i * P:(i + 1) * P, :], out_t)


def build_kernel(CH, SDEV):
    nc = bacc.Bacc("TRN2", target_bir_lowering=False, debug=False,
                   num_devices=N_CORES)
    smax = max(SDEV)
    args = dict(
        xT_loc=nc.dram_tensor("xT_loc", [D_MODEL, T_LOC], FP32,
                              kind="ExternalInput"),
        x_rows_loc=nc.dram_tensor("x_rows_loc", [T_LOC, D_MODEL], BF16,
                                  kind="ExternalInput"),
        gate_w=nc.dram_tensor("gate_w", [D_MODEL, N_EXPERTS], FP32,
                              kind="ExternalInput"),
        gate_b_rep=nc.dram_tensor("gate_b_rep", [P, N_EXPERTS], FP32,
                                  kind="ExternalInput"),
        iota8_rep=nc.dram_tensor("iota8_rep", [P, N_EXPERTS], FP32,
                                 kind="ExternalInput"),
        w1_loc=nc.dram_tensor("w1_loc", [D_MODEL, HIDDEN], BF16,
                              kind="ExternalInput"),
        b1_loc=nc.dram_tensor("b1_loc", [P, H_BLKS], FP32,
                              kind="ExternalInput"),
        w2_loc=nc.dram_tensor("w2_loc", [HIDDEN, D_MODEL], BF16,
                              kind="ExternalInput"),
        b2_rep=nc.dram_tensor("b2_rep", [P, D_MODEL], FP32,
                              kind="ExternalInput"),
        riota_w=nc.dram_tensor("riota_w", [P, smax // 16], FP32,
                               kind="ExternalInput"),
        idid16=nc.dram_tensor("idid16", [P, N_EXPERTS * (CH[0] + 1) // 16],
                              mybir.dt.int16, kind="ExternalInput"),
        riota_cols=nc.dram_tensor("riota_cols", [P, smax // P], FP32,
                                  kind="ExternalInput"),
        out_loc=nc.dram_tensor("out_loc", [T_LOC, D_MODEL], FP32,
                               kind="ExternalOutput"),
    )
    with tile.TileContext(nc) as tc:
        _body(tc, CH, SDEV, **{k: v.ap() for k, v in args.items()})
    nc.compile()
    return nc


def _capacity(flat_x, gate_w, gate_b):
    """Host gating for compile-time capacities, with margin against tiny fp
    reorder flips between host and device gating."""
    logits = flat_x @ gate_w + gate_b
    top2 = np.argsort(-logits, axis=1, kind="stable")[:, :TOP_K]
    blocks = top2.reshape(N_CORES, N_TOK_TILES, P, TOP_K)
    CH, SDEV = [], []
    for gi, (base, HB) in enumerate(GROUPS):
        blk = blocks[:, base:base + HB]
        counts = np.stack([(blk == e).sum(axis=(1, 2, 3))
                           for e in range(N_EXPERTS)])  # [E, src]
        C = int(counts.max()) + 8
        if gi == 0:
            # group 0 computes the full padded recv buffer (no compaction),
            # so its chunk rows must make SROWS a multiple of 128
            C = ((C + 1 + 15) // 16) * 16 - 1
            S = N_EXPERTS * (C + 1)
        else:
            S = ((int(counts.sum(axis=1).max()) + 16 + 127) // 128) * 128
        CH.append(C)
        SDEV.append(S)
    assert max(CH) + 1 < 256  # counts must stay bf16-exact
    return tuple(CH), tuple(SDEV)


_CACHE = {}


def kernel(x, gate_w, gate_b, w1, b1, w2, b2, _trace=False):
    x = np.ascontiguousarray(np.asarray(x, dtype=np.float32))
    gate_w = np.ascontiguousarray(np.asarray(gate_w, dtype=np.float32))
    gate_b = np.ascontiguousarray(np.asarray(gate_b, dtype=np.float32))
    w1 = np.asarray(w1, dtype=np.float32)
    b1 = np.asarray(b1, dtype=np.float32)
    w2 = np.asarray(w2, dtype=np.float32)
    b2 = np.asarray(b2, dtype=np.float32)

    orig_shape = x.shape
    flat_x = x.reshape(-1, D_MODEL)
    CH, SDEV = _capacity(flat_x, gate_w, gate_b)

    if (CH, SDEV) not in _CACHE:
        _CACHE[(CH, SDEV)] = build_kernel(CH, SDEV)
    nc = _CACHE[(CH, SDEV)]

    smax = max(SDEV)
    iota8 = np.tile(np.arange(N_EXPERTS, dtype=np.float32), (P, 1))
    # wrapped iota (idx k lives at [k%16, k//16]), replicated to 128 parts
    riota_w = np.tile(
        (np.arange(16, dtype=np.float32)[:, None]
         + 16.0 * np.arange(smax // 16, dtype=np.float32)[None, :]), (8, 1))
    riota_cols = np.ascontiguousarray(
        (np.arange(P, dtype=np.float32)[:, None]
         + float(P) * np.arange(smax // P, dtype=np.float32)[None, :]))
    s0 = N_EXPERTS * (CH[0] + 1)
    idid16 = np.ascontiguousarray(np.tile(
        (np.arange(16, dtype=np.int16)[:, None]
         + 16 * np.arange(s0 // 16, dtype=np.int16)[None, :]), (8, 1)))
    gb_rep = np.tile(gate_b, (P, 1))
    w1_bf = w1.astype(ml_dtypes.bfloat16)
    w2_bf = w2.astype(ml_dtypes.bfloat16)
    x_bf = flat_x.astype(ml_dtypes.bfloat16)
    in_maps = []
    for c in range(N_CORES):
        in_maps.append({
            "xT_loc": np.ascontiguousarray(
                flat_x[c * T_LOC:(c + 1) * T_LOC].T),
            "x_rows_loc": np.ascontiguousarray(
                x_bf[c * T_LOC:(c + 1) * T_LOC]),
            "gate_w": gate_w,
            "gate_b_rep": gb_rep,
            "iota8_rep": iota8,
            "w1_loc": np.ascontiguousarray(w1_bf[c]),
            "b1_loc": np.ascontiguousarray(b1[c].reshape(H_BLKS, P).T),
            "w2_loc": np.ascontiguousarray(w2_bf[c]),
            "b2_rep": np.tile(b2[c], (P, 1)),
            "riota_w": np.ascontiguousarray(riota_w),
            "idid16": idid16,
            "riota_cols": riota_cols,
        })

    res = run_bass_kernel_spmd(nc, in_maps, core_ids=list(range(N_CORES)),
                               trace=_trace)
    out = np.concatenate([res.results[c]["out_loc"] for c in range(N_CORES)],
                         axis=0)
    if _trace:
        kernel.last_results = res
    return out.reshape(orig_shape)


# revision 18
# speedup vs baseline: 1.0332x; 1.0332x over previous
"""MoE feed-forward (8 experts, top-2) on 8 TRN2 NeuronCores, expert-parallel.

v2.2: latency-oriented restructure of the 956us baseline.

Per-core pipeline (core c owns expert c):
  A. fp32 gating from host-pretransposed xT (gate_w-stationary PE matmuls),
     top-2 via DVE max/max_index, sigmoid weights; per-group triangular-
     matmul prefix sums assign [expert, C_g+1] send slots; bf16 x rows come
     pre-cast from the host. xT streams in first across four DMA queues so
     gating starts ~12us in; expert weights queue behind it.
  B. Dispatch split into two AllToAlls over token groups (0-255, 256-1023):
     the small first group fires ~25us in, absorbing inter-core start skew,
     and its chunk row C_g carries the per-expert counts so the receive-side
     slot tables build with no extra collective.
  C. Expert MLP per 512-row tile, fed by dma_gather(transpose=True) straight
     from the padded recv buffer into xrT [128, d-blk, tok] bf16 layout.
     w1 runs two interleaved PSUM accumulation chains (m-block pairs) and w2
     alternates its two d-chunk banks per m so no two consecutive matmuls
     accumulate into the same PSUM bank.
  D. y rows scatter (indirect, OOB-clipped) back into recv-slot order; one
     return A2A per group so most of the return wire time hides under
     compute.
  E. Combine: indirect gathers of each token's two expert rows + weighted
     add, fp32 out.
"""
import numpy as np
import ml_dtypes

import concourse.bass as bass
import concourse.mybir as mybir
import concourse.tile as tile
from concourse import bacc
from concourse.bass import IndirectOffsetOnAxis
from concourse.bass_utils import run_bass_kernel_spmd
from concourse.masks import make_identity, make_upper_triangular

D_MODEL, HIDDEN, N_EXPERTS, TOP_K = 1024, 4096, 8, 2
N_CORES = 8
P = 128
T = 8192
T_LOC = T // N_CORES            # 1024 tokens per core
N_TOK_TILES = T_LOC // P        # 8
D_BLKS = D_MODEL // P           # 8
H_BLKS = HIDDEN // P            # 32
GROUPS = [(0, 2), (2, 5), (7, 1)]   # (first token tile, n tiles) per group

FP32 = mybir.dt.float32
BF16 = mybir.dt.bfloat16
I32 = mybir.dt.int32
U32 = mybir.dt.uint32
AF = mybir.ActivationFunctionType
ALU = mybir.AluOpType

RG = [list(range(N_CORES))]


def _dram_alias(nc, base_handle, name):
    """A DRAM tensor handle aliasing base_handle's memory. Distinct names keep
    Tile's conservative same-tensor WAW tracking from serializing writers that
    are known (by construction) to touch disjoint rows."""
    mls = nc._tensor(name, list(base_handle.shape), base_handle.dtype,
                     kind="Internal", type="DRAM")
    base_mloc = nc.lookup_mloc(base_handle)
    mloc = mls.memorylocations[0]
    mloc.allocated = base_mloc.allocated
    mloc.addr = base_mloc.addr
    return bass.DRamTensorHandle(name, list(base_handle.shape),
                                 base_handle.dtype)


def _tiles_of(total, big=512):
    out = []
    off = 0
    while off < total:
        nt = min(big, total - off)
        out.append((off, nt))
        off += nt
    return out


def _body(tc, CH, SDEV, xT_loc, x_rows_loc, gate_w, gate_b_rep, iota8_rep,
          w1_loc, b1_loc, w2_loc, b2_rep, riota_w, riota_cols, out_loc):
    """CH = per-group slot capacities (chunk rows are C+1; row C is counts).
    SDEV = per-group 128-multiple compute row counts."""
    nc = tc.nc
    NG = len(GROUPS)
    SROWS = [N_EXPERTS * (c + 1) for c in CH]

    send_x_t = [nc.dram_tensor(f"send_x{h}", [SROWS[h], D_MODEL], BF16)
                for h in range(NG)]
    send_x_al = [[_dram_alias(nc, send_x_t[h], f"sx{h}_al{i}")
                  for i in range(GROUPS[h][1] * TOP_K + 1)] for h in range(NG)]
    send_y_t = [nc.dram_tensor(f"send_y{h}", [SROWS[h], D_MODEL], BF16)
                for h in range(NG)]
    send_y_al = [[_dram_alias(nc, send_y_t[h], f"sy{h}_al{i}")
                  for i in range(SDEV[h] // P)] for h in range(NG)]

    with tc.tile_pool(name="dram", bufs=1, space="DRAM") as dram, \
         tc.tile_pool(name="persist", bufs=1) as persist:
        recv_x = [dram.tile([SROWS[h], D_MODEL], BF16, name=f"recv_x{h}",
                            tag=f"recv_x{h}") for h in range(NG)]
        recv_y = [dram.tile([SROWS[h], D_MODEL], BF16, name=f"recv_y{h}",
                            tag=f"recv_y{h}") for h in range(NG)]

        ident = persist.tile([P, P], FP32)
        make_identity(nc, ident)
        strictu = persist.tile([P, P], FP32)
        make_upper_triangular(nc, strictu, val=1.0, diag=False)
        ones_t = persist.tile([P, P], FP32)
        nc.gpsimd.memset(ones_t, 1.0)

        gb_sb = persist.tile([P, N_EXPERTS], FP32)
        nc.scalar.dma_start(gb_sb, gate_b_rep[:])
        iota_sb = persist.tile([P, N_EXPERTS], FP32)
        nc.scalar.dma_start(iota_sb, iota8_rep[:])
        gw_sb = persist.tile([P, D_BLKS, N_EXPERTS], FP32)
        nc.scalar.dma_start(gw_sb, gate_w[:].rearrange("(j p) e -> p j e", p=P))
        b1_sb = persist.tile([P, H_BLKS], FP32)
        nc.scalar.dma_start(b1_sb, b1_loc[:])
        b2r_sb = persist.tile([P, D_MODEL], FP32)
        nc.scalar.dma_start(b2r_sb, b2_rep[:])
        riw_sb = persist.tile([P, max(SDEV) // 16], FP32)
        nc.scalar.dma_start(riw_sb, riota_w[:])
        ric_sb = persist.tile([P, max(SDEV) // P], FP32)
        nc.scalar.dma_start(ric_sb, riota_cols[:])

        wts_sb = persist.tile([P, N_TOK_TILES, TOP_K], FP32)
        rows32 = persist.tile([P, N_TOK_TILES, TOP_K], I32)
        sendmask = persist.tile([P, N_TOK_TILES * N_EXPERTS], FP32)
        gth_sb = persist.tile([P, NG, max(SDEV) // 16], mybir.dt.int16)
        ret_sb = persist.tile([P, NG, max(SDEV) // P], I32)

        w1_sb = persist.tile([P, D_BLKS, HIDDEN], BF16)
        w2_sb = persist.tile([P, H_BLKS, D_MODEL], BF16)

        # =========== phase A: gating + routing + dispatch scatter ===========
        a2a_x = []
        with tc.tile_pool(name="phA", bufs=2) as pA, \
             tc.tile_pool(name="phA1", bufs=1) as pA1, \
             tc.tile_pool(name="phA_psum", bufs=2, space="PSUM") as pAp:
            # xT first, split across the three DMA queues, so gating starts
            # early; x row copies next; the 16MB weight stream queues after.
            qs = [nc.scalar, nc.gpsimd, nc.sync]
            xT_sb = pA1.tile([P, D_BLKS, T_LOC], FP32)
            xT_insts = []
            for j in range(D_BLKS):
                xi = qs[j % 3].dma_start(xT_sb[:, j, :],
                                         xT_loc[j * P:(j + 1) * P, :])
                xT_insts.append(xi)
            x_rows = pA1.tile([P, N_TOK_TILES, D_MODEL], BF16)
            for i in range(N_TOK_TILES):
                qs[i % 3].dma_start(x_rows[:, i, :],
                                    x_rows_loc[i * P:(i + 1) * P, :])
            for j in range(D_BLKS):
                wi = nc.sync.dma_start(w1_sb[:, j, :],
                                       w1_loc[j * P:(j + 1) * P, :])
                if j == 0:
                    for xi in xT_insts:
                        bass._add_dep_helper(wi.ins, xi.ins, sync=True,
                                             reason="weights after xT")
            for m in range(H_BLKS):
                nc.sync.dma_start(w2_sb[:, m, :],
                                  w2_loc[m * P:(m + 1) * P, :])

            logitc = pA1.tile([P, N_TOK_TILES, N_EXPERTS], FP32)
            maxcat = pA1.tile([P, N_TOK_TILES, 8], FP32)
            idxcat8 = pA1.tile([P, N_TOK_TILES, 8], U32)
            idxfcat = pA1.tile([P, N_TOK_TILES, TOP_K], FP32)
            offs = pA1.tile([P, N_TOK_TILES, N_EXPERTS], FP32)
            csum_sb = pA1.tile([P, N_TOK_TILES * N_EXPERTS], FP32)
            gated = [False, False]

            def emit_gate_chunk(tc2):
                gps = pAp.tile([8, 512], FP32, tag="gps", name="gps")
                for j in range(D_BLKS):
                    nc.tensor.matmul(gps, lhsT=gw_sb[:, j, :],
                                     rhs=xT_sb[:, j, tc2 * 512:(tc2 + 1) * 512],
                                     start=(j == 0), stop=(j == D_BLKS - 1))
                logit8 = pA.tile([8, 512], FP32, tag="lg8", name="lg8")
                nc.vector.tensor_copy(logit8, gps)
                gated[tc2] = logit8

            def emit_tile_top2(i):
                logit8 = gated[i // 4]
                ii = i % 4
                tp = pAp.tile([P, 8], FP32, tag="tp", name="tp", bufs=2)
                nc.tensor.transpose(tp, logit8[:, ii * P:(ii + 1) * P],
                                    ident[0:8, 0:8])
                nc.vector.tensor_add(logitc[:, i, :], tp, gb_sb)
                nc.vector.max(maxcat[:, i, :], logitc[:, i, :])
                nc.vector.max_index(idxcat8[:, i, :], maxcat[:, i, :],
                                    logitc[:, i, :])

            emit_gate_chunk(0)
            for i in range(0, 4):
                emit_tile_top2(i)

            for h, (base, HB) in enumerate(GROUPS):
                C = CH[h]
                sl = slice(base * N_EXPERTS, (base + HB) * N_EXPERTS)
                hsl = slice(base, base + HB)

                d01 = pA.tile([P, HB], FP32, tag=f"d01{h}", name=f"d01{h}")
                nc.vector.tensor_sub(d01, maxcat[:, hsl, 0],
                                     maxcat[:, hsl, 1])
                nc.scalar.activation(wts_sb[:, hsl, 0], d01, AF.Sigmoid)
                nc.scalar.activation(wts_sb[:, hsl, 1], d01, AF.Sigmoid,
                                     scale=-1.0)
                nc.vector.tensor_copy(idxfcat[:, hsl, :],
                                      idxcat8[:, hsl, 0:TOP_K])
                eqc = [None, None]
                for k in range(TOP_K):
                    eqk = pA.tile([P, HB, N_EXPERTS], FP32, tag=f"eq{k}{h}",
                                  name=f"eq{k}{h}")
                    nc.vector.tensor_tensor(
                        eqk,
                        idxfcat[:, hsl, k:k + 1].to_broadcast(
                            [P, HB, N_EXPERTS]),
                        iota_sb[:, None, :].to_broadcast([P, HB, N_EXPERTS]),
                        op=ALU.is_equal)
                    eqc[k] = eqk
                nc.vector.tensor_add(
                    sendmask[:].rearrange("p (i e) -> p i e", e=N_EXPERTS)
                    [:, hsl, :], eqc[0], eqc[1])

                pref_ps = pAp.tile([P, HB * N_EXPERTS], FP32, tag="pref",
                                   name="pref", bufs=2)
                nc.tensor.matmul(pref_ps, lhsT=strictu, rhs=sendmask[:, sl],
                                 start=True, stop=True)
                csum_ps = pAp.tile([P, HB * N_EXPERTS], FP32, tag="csum",
                                   name="csum", bufs=2)
                nc.tensor.matmul(csum_ps, lhsT=ones_t, rhs=sendmask[:, sl],
                                 start=True, stop=True)
                offs_flat = offs[:].rearrange("p a b -> p (a b)")
                nc.vector.tensor_copy(offs_flat[:, sl], pref_ps)
                nc.vector.tensor_copy(csum_sb[:, sl], csum_ps)
                for i in range(base + 1, base + HB):
                    prev = csum_sb[:, (i - 1) * N_EXPERTS:i * N_EXPERTS]
                    nc.vector.tensor_add(offs[:, i, :], offs[:, i, :], prev)
                    cur = csum_sb[:, i * N_EXPERTS:(i + 1) * N_EXPERTS]
                    nc.vector.tensor_add(cur, cur, prev)

                rowsf = pA.tile([P, HB, TOP_K], FP32, tag=f"rowsf{h}",
                                name=f"rowsf{h}")
                nc.vector.tensor_scalar(rowsf, idxfcat[:, hsl, :],
                                        float(C + 1), None, op0=ALU.mult)
                for k in range(TOP_K):
                    prod = pA.tile([P, HB, N_EXPERTS], FP32, tag=f"prod{h}",
                                   name=f"prod{h}")
                    nc.vector.tensor_mul(prod, offs[:, hsl, :], eqc[k])
                    slotk = pA.tile([P, HB], FP32, tag=f"slotk{h}",
                                    name=f"slotk{h}")
                    nc.vector.reduce_sum(slotk, prod,
                                         axis=mybir.AxisListType.X)
                    nc.vector.tensor_add(rowsf[:, :, k], rowsf[:, :, k],
                                         slotk)
                nc.vector.tensor_copy(rows32[:, hsl, :], rowsf)

                # counts -> row C of every expert chunk (16 bf16 elems each)
                cntw = pA.tile([1, N_EXPERTS, 16], BF16, tag=f"cntw{h}",
                               name=f"cntw{h}")
                last = csum_sb[0:1, (base + HB - 1) * N_EXPERTS:
                               (base + HB) * N_EXPERTS]
                nc.vector.tensor_copy(
                    cntw, last[:, :, None].to_broadcast([1, N_EXPERTS, 16]))
                scatter_insts = []
                si = nc.scalar.dma_start(
                    send_x_al[h][HB * TOP_K].ap()[:]
                    .rearrange("(e r) d -> e r d", r=C + 1)[:, C, 0:16],
                    cntw[:])
                scatter_insts.append(si)

                for ii in range(HB):
                    i = base + ii
                    for k in range(TOP_K):
                        si = nc.gpsimd.indirect_dma_start(
                            out=send_x_al[h][ii * TOP_K + k].ap(),
                            out_offset=IndirectOffsetOnAxis(
                                ap=rows32[:, i, k:k + 1], axis=0),
                            in_=x_rows[:, i, :],
                            in_offset=None,
                            bounds_check=SROWS[h] - 1, oob_is_err=False)
                        scatter_insts.append(si)

                if h < NG - 1:
                    cc = nc.gpsimd.collective_compute(
                        "AllToAll", ALU.bypass, replica_groups=RG,
                        ins=[send_x_t[h].ap()[:].opt()],
                        outs=[recv_x[h][:].opt()])
                    for si in scatter_insts:
                        bass._add_dep_helper(cc.ins, si.ins, sync=True,
                                             reason="a2a after aliased scatters")
                    a2a_x.append(cc)
                else:
                    last_scatters = scatter_insts

                if h == 0:
                    emit_gate_chunk(1)
                    for i in range(2, N_TOK_TILES):
                        emit_tile_top2(i)

        # ====== phases B-D: per-group slot tables + expert MLP + return ======
        a2a_y = []
        with tc.tile_pool(name="phC", bufs=2) as pC, \
             tc.tile_pool(name="phCh", bufs=1) as pCh, \
             tc.tile_pool(name="phC_psum", bufs=4, space="PSUM") as pCp:
            hT = pCh.tile([P, H_BLKS, 512], BF16)
            early_gathers = []

            _g0ctx = {}

            def emit_rc0(NCOL):
                C, S, cntb = _g0ctx["C"], _g0ctx["S"], _g0ctx["cntb"]
                msk3 = pC.tile([P, NCOL, N_EXPERTS], FP32, tag="rc0m3",
                               name="rc0m3", bufs=1)
                nc.vector.tensor_tensor(
                    msk3,
                    ric_sb[:, 0:NCOL, None].to_broadcast(
                        [P, NCOL, N_EXPERTS]),
                    cntb[:, None, N_EXPERTS:2 * N_EXPERTS]
                    .to_broadcast([P, NCOL, N_EXPERTS]),
                    op=ALU.is_ge)
                s_of = pC.tile([P, NCOL], FP32, tag="rc0s", name="rc0s",
                               bufs=1)
                nc.vector.reduce_sum(s_of, msk3[:],
                                     axis=mybir.AxisListType.X)
                nc.vector.tensor_tensor(
                    msk3, msk3[:],
                    cntb[:, None, 0:N_EXPERTS].to_broadcast(
                        [P, NCOL, N_EXPERTS]),
                    op=ALU.mult)
                cume = pC.tile([P, NCOL], FP32, tag="rc0c", name="rc0c",
                               bufs=1)
                nc.vector.reduce_sum(cume, msk3[:],
                                     axis=mybir.AxisListType.X)
                gf = pC.tile([P, NCOL], FP32, tag="rc0g", name="rc0g",
                             bufs=1)
                nc.vector.tensor_scalar(gf, s_of, float(C + 1), None,
                                        op0=ALU.mult)
                nc.vector.tensor_add(gf, gf, ric_sb[:, 0:NCOL])
                nc.vector.tensor_sub(gf, gf, cume)
                nc.vector.tensor_copy(ret_sb[:, 0, 0:NCOL], gf)

            def emit_table(h, q):
                C = CH[h]
                S = SROWS[h]
                NW = SDEV[h] // 16
                NCOL = SDEV[h] // P
                cnt128 = pC.tile([1, N_EXPERTS * 16], BF16, tag="cnt128",
                                 name="cnt128", bufs=1)
                q.dma_start(
                    cnt128,
                    recv_x[h][:].rearrange("(e r) d -> e r d", r=C + 1)
                    [:, C, 0:16])
                cnt2 = pC.tile([1, 2 * N_EXPERTS], FP32, tag="cnt2",
                               name="cnt2", bufs=1)
                cnt_row = cnt2[:, 0:N_EXPERTS]
                cum_row = cnt2[:, N_EXPERTS:]
                cnt_v = cnt128[:].rearrange("o (e u) -> o e u", u=16)[:, :, 0]
                nc.vector.tensor_copy(cnt_row, cnt_v)
                nc.vector.tensor_copy(cum_row, cnt_v)
                for s in range(1, N_EXPERTS):
                    nc.vector.tensor_add(cum_row[:, s:s + 1],
                                         cum_row[:, s:s + 1],
                                         cum_row[:, s - 1:s])
                bc = pCp.tile([P, 2 * N_EXPERTS], FP32, tag="bc", name="bc",
                              bufs=1)
                nc.tensor.matmul(bc, lhsT=ones_t[0:1, :], rhs=cnt2[:],
                                 start=True, stop=True)
                cntb = pC.tile([P, 2 * N_EXPERTS], FP32, tag="cntb",
                               name="cntb", bufs=1)
                nc.vector.tensor_copy(cntb, bc)
                if h == 0:
                    _g0ctx.clear()
                    _g0ctx.update(C=C, S=S, cntb=cntb)

                def slot_table(out_ap, iota_ap, np_, tag, clamp):
                    msk3 = pC.tile([P, np_, N_EXPERTS], FP32, tag=tag + "m3",
                                   name=tag + "m3", bufs=1)
                    nc.vector.tensor_tensor(
                        msk3,
                        iota_ap[:, :, None].to_broadcast([P, np_, N_EXPERTS]),
                        cntb[:, None, N_EXPERTS:2 * N_EXPERTS]
                        .to_broadcast([P, np_, N_EXPERTS]),
                        op=ALU.is_ge)
                    s_of = pC.tile([P, np_], FP32, tag=tag + "s",
                                   name=tag + "s", bufs=1)
                    nc.vector.reduce_sum(s_of, msk3[:],
                                         axis=mybir.AxisListType.X)
                    nc.vector.tensor_tensor(
                        msk3, msk3[:],
                        cntb[:, None, 0:N_EXPERTS].to_broadcast(
                            [P, np_, N_EXPERTS]),
                        op=ALU.mult)
                    cume = pC.tile([P, np_], FP32, tag=tag + "c",
                                   name=tag + "c", bufs=1)
                    nc.vector.reduce_sum(cume, msk3[:],
                                         axis=mybir.AxisListType.X)
                    gf = pC.tile([P, np_], FP32, tag=tag + "g",
                                 name=tag + "g", bufs=1)
                    nc.vector.tensor_scalar(gf, s_of, float(C + 1), None,
                                            op0=ALU.mult)
                    nc.vector.tensor_add(gf, gf, iota_ap)
                    nc.vector.tensor_sub(gf, gf, cume)
                    if clamp:
                        nc.vector.tensor_scalar(gf, gf, float(S - 1), None,
                                                op0=ALU.min)
                    nc.vector.tensor_copy(out_ap, gf)

                slot_table(gth_sb[:, h, 0:NW], riw_sb[:, 0:NW], NW,
                           f"gw{h}", True)
                if h > 0:
                    # later groups: ret table built here too (off any path)
                    slot_table(ret_sb[:, h, 0:NCOL], ric_sb[:, 0:NCOL],
                               NCOL, f"rc{h}", False)

            for h in range(NG):
                C = CH[h]
                S = SROWS[h]
                NCOL = SDEV[h] // P
                if h == 0:
                    emit_table(0, nc.scalar)

                ret_insts = []
                for ti, (r0, NT) in enumerate(_tiles_of(SDEV[h])):
                    xrT = pC.tile([P, D_BLKS, NT], BF16, tag=f"xrT{NT}",
                                  name=f"xrT{NT}", bufs=2 if NT == 512 else 1)
                    gi = nc.gpsimd.dma_gather(
                        xrT[:], recv_x[h][:],
                        gth_sb[:, h, r0 // 16:(r0 + NT) // 16],
                        NT, NT, D_MODEL, transpose=True)
                    if h == 0 and ti <= 1:
                        early_gathers.append(gi)
                    if h == 0 and ti == 1:
                        # last group's dispatch A2A, deferred so its
                        # completion-serialized trigger doesn't block the
                        # first compute gathers on the gpsimd queue
                        ccl = nc.gpsimd.collective_compute(
                            "AllToAll", ALU.bypass, replica_groups=RG,
                            ins=[send_x_t[NG - 1].ap()[:].opt()],
                            outs=[recv_x[NG - 1][:].opt()])
                        for si in last_scatters:
                            bass._add_dep_helper(ccl.ins, si.ins, sync=True,
                                                 reason="a2a after scatters")
                        for gi2 in early_gathers:
                            bass._add_dep_helper(ccl.ins, gi2.ins, sync=True,
                                                 reason="trigger after gathers")
                        a2a_x.append(ccl)
                    for mp in range(0, H_BLKS, 2):
                        psA = pCp.tile([P, 512], FP32, tag="ps1",
                                       name="ps1", bufs=4)
                        psB = pCp.tile([P, 512], FP32, tag="ps1",
                                       name="ps1", bufs=4)
                        for j in range(D_BLKS):
                            nc.tensor.matmul(
                                psA[:, 0:NT],
                                lhsT=w1_sb[:, j, mp * P:(mp + 1) * P],
                                rhs=xrT[:, j, :],
                                start=(j == 0), stop=(j == D_BLKS - 1),
                                skip_group_check=True)
                            nc.tensor.matmul(
                                psB[:, 0:NT],
                                lhsT=w1_sb[:, j, (mp + 1) * P:(mp + 2) * P],
                                rhs=xrT[:, j, :],
                                start=(j == 0), stop=(j == D_BLKS - 1),
                                skip_group_check=True)
                        nc.scalar.activation(hT[:, mp, 0:NT], psA[:, 0:NT],
                                             AF.Silu, bias=b1_sb[:, mp:mp + 1])
                        nc.scalar.activation(hT[:, mp + 1, 0:NT],
                                             psB[:, 0:NT], AF.Silu,
                                             bias=b1_sb[:, mp + 1:mp + 2])
                    if ti == 0:
                        if h == 0:
                            # group-0 ret table off the critical path
                            emit_rc0(NCOL)
                        if h + 1 < NG:
                            # next group's tables build during this group
                            emit_table(h + 1, nc.sync)
                    for t in range(NT // P):
                        col = r0 // P + t
                        y_tm = pC.tile([P, D_MODEL], BF16, tag="y_tm",
                                       name="y_tm", bufs=2)
                        ps2 = [pCp.tile([P, 512], FP32, tag="ps2",
                                        name="ps2", bufs=2)
                               for _ in range(2)]
                        for m in range(H_BLKS):
                            for nh in range(2):
                                nc.tensor.matmul(
                                    ps2[nh], lhsT=hT[:, m, t * P:(t + 1) * P],
                                    rhs=w2_sb[:, m, nh * 512:(nh + 1) * 512],
                                    start=(m == 0), stop=(m == H_BLKS - 1),
                                    skip_group_check=True)
                        for nh in range(2):
                            nc.vector.tensor_add(
                                y_tm[:, nh * 512:(nh + 1) * 512], ps2[nh],
                                b2r_sb[:, nh * 512:(nh + 1) * 512])
                        si = nc.gpsimd.indirect_dma_start(
                            out=send_y_al[h][col].ap(),
                            out_offset=IndirectOffsetOnAxis(
                                ap=ret_sb[:, h, col:col + 1], axis=0),
                            in_=y_tm[:],
                            in_offset=None,
                            bounds_check=S - 1, oob_is_err=False)
                        ret_insts.append(si)

                cc = nc.gpsimd.collective_compute(
                    "AllToAll", ALU.bypass, replica_groups=RG,
                    ins=[send_y_t[h].ap()[:].opt()],
                    outs=[recv_y[h][:].opt()])
                for si in ret_insts:
                    bass._add_dep_helper(cc.ins, si.ins, sync=True,
                                         reason="ret a2a after aliased scatters")
                a2a_y.append(cc)

        # =========== phase E: gather + weighted combine ===========
        with tc.tile_pool(name="phE", bufs=2) as pE:
            for h, (base, HB) in enumerate(GROUPS):
                for i in range(base, base + HB):
                    g0 = pE.tile([P, D_MODEL], BF16, tag="g0", name="g0")
                    nc.gpsimd.indirect_dma_start(
                        out=g0, out_offset=None, in_=recv_y[h][:],
                        in_offset=IndirectOffsetOnAxis(ap=rows32[:, i, 0:1],
                                                       axis=0))
                    g1 = pE.tile([P, D_MODEL], BF16, tag="g1", name="g1")
                    nc.gpsimd.indirect_dma_start(
                        out=g1, out_offset=None, in_=recv_y[h][:],
                        in_offset=IndirectOffsetOnAxis(ap=rows32[:, i, 1:2],
                                                       axis=0))
                    t0 = pE.tile([P, D_MODEL], FP32, tag="t0", name="t0")
                    nc.vector.tensor_scalar_mul(t0, g0, wts_sb[:, i, 0:1])
                    t1 = pE.tile([P, D_MODEL], FP32, tag="t1", name="t1")
                    nc.vector.tensor_scalar_mul(t1, g1, wts_sb[:, i, 1:2])
                    out_t = pE.tile([P, D_MODEL], FP32, tag="out_t",
                                    name="out_t")
                    nc.vector.tensor_add(out_t, t0, t1)
                    nc.scalar.dma_start(out_loc[i * P:(i + 1) * P, :], out_t)


def build_kernel(CH, SDEV):
    nc = bacc.Bacc("TRN2", target_bir_lowering=False, debug=False,
                   num_devices=N_CORES)
    smax = max(SDEV)
    args = dict(
        xT_loc=nc.dram_tensor("xT_loc", [D_MODEL, T_LOC], FP32,
                              kind="ExternalInput"),
        x_rows_loc=nc.dram_tensor("x_rows_loc", [T_LOC, D_MODEL], BF16,
                                  kind="ExternalInput"),
        gate_w=nc.dram_tensor("gate_w", [D_MODEL, N_EXPERTS], FP32,
                              kind="ExternalInput"),
        gate_b_rep=nc.dram_tensor("gate_b_rep", [P, N_EXPERTS], FP32,
                                  kind="ExternalInput"),
        iota8_rep=nc.dram_tensor("iota8_rep", [P, N_EXPERTS], FP32,
                                 kind="ExternalInput"),
        w1_loc=nc.dram_tensor("w1_loc", [D_MODEL, HIDDEN], BF16,
                              kind="ExternalInput"),
        b1_loc=nc.dram_tensor("b1_loc", [P, H_BLKS], FP32,
                              kind="ExternalInput"),
        w2_loc=nc.dram_tensor("w2_loc", [HIDDEN, D_MODEL], BF16,
                              kind="ExternalInput"),
        b2_rep=nc.dram_tensor("b2_rep", [P, D_MODEL], FP32,
                              kind="ExternalInput"),
        riota_w=nc.dram_tensor("riota_w", [P, smax // 16], FP32,
                               kind="ExternalInput"),
        riota_cols=nc.dram_tensor("riota_cols", [P, smax // P], FP32,
                                  kind="ExternalInput"),
        out_loc=nc.dram_tensor("out_loc", [T_LOC, D_MODEL], FP32,
                               kind="ExternalOutput"),
    )
    with tile.TileContext(nc) as tc:
        _body(tc, CH, SDEV, **{k: v.ap() for k, v in args.items()})
    nc.compile()
    return nc


def _capacity(flat_x, gate_w, gate_b):
    """Host gating for compile-time capacities, with margin against tiny fp
    reorder flips between host and device gating."""
    logits = flat_x @ gate_w + gate_b
    top2 = np.argsort(-logits, axis=1, kind="stable")[:, :TOP_K]
    blocks = top2.reshape(N_CORES, N_TOK_TILES, P, TOP_K)
    CH, SDEV = [], []
    for (base, HB) in GROUPS:
        blk = blocks[:, base:base + HB]
        counts = np.stack([(blk == e).sum(axis=(1, 2, 3))
                           for e in range(N_EXPERTS)])  # [E, src]
        C = int(counts.max()) + 8
        S = ((int(counts.sum(axis=1).max()) + 16 + 127) // 128) * 128
        CH.append(C)
        SDEV.append(S)
    assert max(CH) + 1 < 256  # counts must stay bf16-exact
    return tuple(CH), tuple(SDEV)


_CACHE = {}


def kernel(x, gate_w, gate_b, w1, b1, w2, b2, _trace=False):
    x = np.ascontiguousarray(np.asarray(x, dtype=np.float32))
    gate_w = np.ascontiguousarray(np.asarray(gate_w, dtype=np.float32))
    gate_b = np.ascontiguousarray(np.asarray(gate_b, dtype=np.float32))
    w1 = np.asarray(w1, dtype=np.float32)
    b1 = np.asarray(b1, dtype=np.float32)
    w2 = np.asarray(w2, dtype=np.float32)
    b2 = np.asarray(b2, dtype=np.float32)

    orig_shape = x.shape
    flat_x = x.reshape(-1, D_MODEL)
    CH, SDEV = _capacity(flat_x, gate_w, gate_b)

    if (CH, SDEV) not in _CACHE:
        _CACHE[(CH, SDEV)] = build_kernel(CH, SDEV)
    nc = _CACHE[(CH, SDEV)]

    smax = max(SDEV)
    iota8 = np.tile(np.arange(N_EXPERTS, dtype=np.float32), (P, 1))
    # wrapped iota (idx k lives at [k%16, k//16]), replicated to 128 parts
    riota_w = np.tile(
        (np.arange(16, dtype=np.float32)[:, None]
         + 16.0 * np.arange(smax // 16, dtype=np.float32)[None, :]), (8, 1))
    riota_cols = np.ascontiguousarray(
        (np.arange(P, dtype=np.float32)[:, None]
         + float(P) * np.arange(smax // P, dtype=np.float32)[None, :]))
    gb_rep = np.tile(gate_b, (P, 1))
    w1_bf = w1.astype(ml_dtypes.bfloat16)
    w2_bf = w2.astype(ml_dtypes.bfloat16)
    x_bf = flat_x.astype(ml_dtypes.bfloat16)
    in_maps = []
    for c in range(N_CORES):
        in_maps.append({
            "xT_loc": np.ascontiguousarray(
                flat_x[c * T_LOC:(c + 1) * T_LOC].T),
            "x_rows_loc": np.ascontiguousarray(
                x_bf[c * T_LOC:(c + 1) * T_LOC]),
            "gate_w": gate_w,
            "gate_b_rep": gb_rep,
            "iota8_rep": iota8,
            "w1_loc": np.ascontiguousarray(w1_bf[c]),
            "b1_loc": np.ascontiguousarray(b1[c].reshape(H_BLKS, P).T),
            "w2_loc": np.ascontiguousarray(w2_bf[c]),
            "b2_rep": np.tile(b2[c], (P, 1)),
            "riota_w": np.ascontiguousarray(riota_w),
            "riota_cols": riota_cols,
        })

    res = run_bass_kernel_spmd(nc, in_maps, core_ids=list(range(N_CORES)),
                               trace=_trace)
    out = np.concatenate([res.results[c]["out_loc"] for c in range(N_CORES)],
                         axis=0)
    if _trace:
        kernel.last_results = res
    return out.reshape(orig_shape)
